# revision 18
# baseline (speedup 1.0000x reference)
"""Trainium2 Bass kernel for nn_APLoss (8 NeuronCores, SPMD row-sharded).

The reference loss collapses to per-row quantities: with c_i = 1 - y_pred[i],
s_ij = relu(c_i + y_pred[j])^2,  R_i = sum_j s_ij,  Rpos_i = sum_{j<2048} s_ij:

  u_a = 0.01*u_all[i] + 0.99*R_i/n        u_p = 0.01*u_pos[i] + 0.99*Rpos_i/n
  loss = mean_i[ (u_p * R_i/n)/u_a^2 - (Rpos_i/n)/u_a ]

Each core owns 256 of the 2048 positive rows (2 partition blocks), holds all of
y_pred, computes its partial sum of the per-row terms; the host sums the 8
partial scalars (the unshard step for a row-sharded scalar mean).

Per-core device pipeline:
  * ScalarE, j in [0, SA): T = Relu(y_j + c_i) with per-partition bias, then
    Square with accum_out -> row sums, split at j=2048 so Rpos falls out.
  * VectorE, j-chunks in [SA, 16384): B[j, r] = (f_r - (y_j+1) < 0) via one
    bf16 tensor_scalar per 128-j chunk (2 elem/cycle/lane).
  * TensorE: (K,S1,S2) = W_chunk^T @ B accumulated in PSUM over chunks (bf16
    in, fp32 acc), W = [1, y, y^2]; then R_tail = (c*K + 2*S1)*c + S2.
  * Small fp32 vector ops do the u-update / p / mean math; PE ones-matmul
    reduces over partitions; each core DMAs out one scalar.

Host-side prep is layout only: dtype casts, reshapes/transposes, the gather
u[index_s[:n_pos]], and replicating y/f slices across the 128 partitions.
"""

import numpy as np

try:
    import concourse.bass as bass  # noqa: F401
except ImportError:  # pragma: no cover
    import sys

    sys.path.insert(0, "/opt/trn_rl_repo")

N = 16384
P = 2048
NCORES = 8
RPC = P // NCORES  # 256 rows per core = 2 partition blocks
SA = 4608  # ScalarE covers j in [0, SA)
K0 = SA // 128  # first indicator j-chunk
ND = (N - SA) // 128  # number of indicator j-chunks
GAMMA = 0.99

_NC_CACHE = {}


def _build_nc():
    import concourse.tile as tile
    from concourse import bacc, mybir

    f32 = mybir.dt.float32
    bf16 = mybir.dt.bfloat16
    Alu = mybir.AluOpType
    Act = mybir.ActivationFunctionType

    nc = bacc.Bacc("TRN2", target_bir_lowering=False, debug=False, num_devices=NCORES)

    y_bc = nc.dram_tensor("y_bc", [128, SA], bf16, kind="ExternalInput").ap()
    f_bc = nc.dram_tensor("f_bc", [128, RPC], bf16, kind="ExternalInput").ap()
    y_cols = nc.dram_tensor("y_cols", [128, 128], f32, kind="ExternalInput").ap()
    f_cols = nc.dram_tensor("f_cols", [128, 2], f32, kind="ExternalInput").ap()
    ua_cols = nc.dram_tensor("ua_cols", [128, 2], f32, kind="ExternalInput").ap()
    up_cols = nc.dram_tensor("up_cols", [128, 2], f32, kind="ExternalInput").ap()
    eye3 = nc.dram_tensor("eye3", [3, 3], f32, kind="ExternalInput").ap()
    ones_col = nc.dram_tensor("ones_col", [128, 1], f32, kind="ExternalInput").ap()
    out = nc.dram_tensor("out", [1, 1], f32, kind="ExternalOutput").ap()

    with tile.TileContext(nc) as tc:
        with (
            tc.tile_pool(name="const", bufs=1) as cpool,
            tc.tile_pool(name="bpool", bufs=8) as bpool,
            tc.tile_pool(name="scratch", bufs=2) as spool,
            tc.tile_pool(name="small", bufs=1) as mpool,
            tc.tile_pool(name="psum", bufs=1, space="PSUM") as ppool,
        ):
            # DMA trigger instructions cost ~0.6us each on the issuing engine,
            # so spread them over three queues: the indicator-path gates
            # (f broadcast + y columns) first on sync, the y broadcast halves
            # on gpsimd/scalar (pos half first).
            # The indicator-path gates (f broadcast + y columns) go alone on
            # the sync HWDGE queue so they get the SDMA engines to themselves.
            Fb = cpool.tile([128, RPC], bf16)
            nc.sync.dma_start(Fb[:], f_bc[:])
            ycols = cpool.tile([128, 128], f32)
            nc.sync.dma_start(ycols[:], y_cols[:])
            # WAW gates: write a corner of Yb from Fb/ycols so the big y
            # broadcast DMAs cannot be scheduled before the gating transfers
            # complete (the SDMA engines round-robin all queued work, which
            # would otherwise starve Fb/ycols).
            Yb = cpool.tile([128, SA], bf16)
            nc.gpsimd.tensor_copy(Yb[0:1, 0:1], Fb[0:1, 0:1])
            nc.scalar.copy(Yb[0:1, 1024:1025], ycols[0:1, 0:1])
            nc.gpsimd.dma_start(Yb[:, 0:1024], y_bc[:, 0:1024])
            nc.scalar.dma_start(Yb[:, 1024:2048], y_bc[:, 1024:2048])
            nc.gpsimd.dma_start(Yb[:, 2048 : (2048 + SA) // 2], y_bc[:, 2048 : (2048 + SA) // 2])
            nc.scalar.dma_start(Yb[:, (2048 + SA) // 2 : SA], y_bc[:, (2048 + SA) // 2 : SA])
            fcols = cpool.tile([128, 2], f32)
            nc.gpsimd.dma_start(fcols[:], f_cols[:])
            ua = cpool.tile([128, 2], f32)
            nc.scalar.dma_start(ua[:], ua_cols[:])
            up = cpool.tile([128, 2], f32)
            nc.scalar.dma_start(up[:], up_cols[:])
            eye3t = cpool.tile([3, 3], f32)
            nc.gpsimd.dma_start(eye3t[:], eye3[:])
            onest = cpool.tile([128, 1], f32)
            nc.gpsimd.dma_start(onest[:], ones_col[:])

            # y + 1 per-chunk bias columns; c = 1 - f per-block bias columns
            y1 = cpool.tile([128, 128], f32)
            nc.vector.tensor_scalar_add(y1[:], ycols[:], 1.0)
            c_cols = cpool.tile([128, 2], f32)
            nc.scalar.activation(c_cols[:], fcols[:], Act.Identity, bias=1.0, scale=-1.0)

            # W[:, t, :] = [1, y, y^2] in bf16 for chunk k = K0 + t
            W = cpool.tile([128, ND, 3], bf16)
            nc.scalar.activation(W[:, :, 0], ycols[:, K0:128], Act.Copy, bias=1.0, scale=0.0)
            nc.vector.tensor_copy(W[:, :, 1], ycols[:, K0:128])
            nc.scalar.activation(W[:, :, 2], ycols[:, K0:128], Act.Square)

            # --- VectorE + TensorE indicator path: j-chunks [K0, 128) ---
            psumQ = ppool.tile([3, RPC], f32)
            for t in range(ND):
                k = K0 + t
                Bt = bpool.tile([128, RPC], bf16, tag="bt")
                nc.vector.tensor_scalar(
                    Bt[:], Fb[:], y1[:, k : k + 1], 0.0, Alu.subtract, Alu.is_lt
                )
                nc.tensor.matmul(
                    psumQ[:], W[:, t, :], Bt[:], start=(t == 0), stop=(t == ND - 1)
                )

            # --- ScalarE path: j in [0, SA), split at the positive boundary ---
            accPos = mpool.tile([128, 2], f32)
            accRest = mpool.tile([128, 2], f32)
            for b in range(2):
                cb = c_cols[:, b : b + 1]
                t1 = spool.tile([128, P], f32, tag="t1")
                nc.scalar.activation(t1[:], Yb[:, 0:P], Act.Relu, bias=cb)
                t2 = spool.tile([128, P], f32, tag="t2")
                nc.scalar.activation(
                    t2[:], t1[:], Act.Square, accum_out=accPos[:, b : b + 1]
                )
                t3 = spool.tile([128, SA - P], f32, tag="t3")
                nc.scalar.activation(t3[:], Yb[:, P:SA], Act.Relu, bias=cb)
                t4 = spool.tile([128, SA - P], f32, tag="t4")
                nc.scalar.activation(
                    t4[:], t3[:], Act.Square, accum_out=accRest[:, b : b + 1]
                )

            # transpose (3, 256) -> per-row (128, 2, 3) via PE with identity;
            # group a is ready halfway through the chunk loop.
            sbQ = mpool.tile([3, RPC], f32)
            nc.scalar.copy(sbQ[:], psumQ[:])
            psumT = ppool.tile([128, 2, 3], f32)
            for h in range(2):
                nc.tensor.matmul(
                    psumT[:, h, :], sbQ[:, h * 128 : (h + 1) * 128], eye3t[:],
                    start=True, stop=True,
                )
            Kq = psumT[:, :, 0]
            S1q = psumT[:, :, 1]
            S2q = psumT[:, :, 2]

            # R_tail = (c*K + 2*S1)*c + S2
            w1 = mpool.tile([128, 2], f32)
            nc.vector.tensor_tensor(w1[:], c_cols[:], Kq, Alu.mult)
            w2 = mpool.tile([128, 2], f32)
            nc.vector.scalar_tensor_tensor(w2[:], S1q, 2.0, w1[:], Alu.mult, Alu.add)
            w3 = mpool.tile([128, 2], f32)
            nc.vector.tensor_tensor(w3[:], w2[:], c_cols[:], Alu.mult)
            Rt = mpool.tile([128, 2], f32)
            nc.vector.tensor_tensor(Rt[:], w3[:], S2q, Alu.add)

            Ra = mpool.tile([128, 2], f32)
            nc.vector.tensor_tensor(Ra[:], accPos[:], accRest[:], Alu.add)
            R = mpool.tile([128, 2], f32)
            nc.vector.tensor_tensor(R[:], Ra[:], Rt[:], Alu.add)

            # u updates, p, and the mean
            uas = mpool.tile([128, 2], f32)
            nc.vector.tensor_scalar_mul(uas[:], ua[:], 1.0 - GAMMA)
            ups = mpool.tile([128, 2], f32)
            nc.vector.tensor_scalar_mul(ups[:], up[:], 1.0 - GAMMA)
            uan = mpool.tile([128, 2], f32)
            nc.vector.scalar_tensor_tensor(uan[:], R[:], GAMMA / N, uas[:], Alu.mult, Alu.add)
            upn = mpool.tile([128, 2], f32)
            nc.vector.scalar_tensor_tensor(
                upn[:], accPos[:], GAMMA / N, ups[:], Alu.mult, Alu.add
            )
            inv = mpool.tile([128, 2], f32)
            nc.vector.reciprocal(inv[:], uan[:])
            g1 = mpool.tile([128, 2], f32)
            nc.vector.tensor_tensor(g1[:], upn[:], R[:], Alu.mult)
            g2 = mpool.tile([128, 2], f32)
            nc.vector.tensor_tensor(g2[:], g1[:], inv[:], Alu.mult)
            g3 = mpool.tile([128, 2], f32)
            nc.vector.tensor_tensor(g3[:], g2[:], accPos[:], Alu.subtract)
            g4 = mpool.tile([128, 2], f32)
            nc.vector.tensor_tensor(g4[:], g3[:], inv[:], Alu.mult)
            acc = mpool.tile([128, 1], f32)
            nc.vector.tensor_reduce(acc[:], g4[:], mybir.AxisListType.X, Alu.add)
            psumF = ppool.tile([1, 1], f32)
            nc.tensor.matmul(psumF[:], onest[:], acc[:], start=True, stop=True)
            outsb = mpool.tile([1, 1], f32)
            # fold the 1/(N*P) mean normalization into the final copy
            nc.scalar.mul(outsb[:], psumF[:], 1.0 / (float(N) * float(P)))
            nc.sync.dma_start(out[:], outsb[:])

    nc.compile()
    return nc


def get_nc():
    if "nc" not in _NC_CACHE:
        _NC_CACHE["nc"] = _build_nc()
    return _NC_CACHE["nc"]


def make_in_maps(y_pred, u_all, u_pos, index_s, n_pos):
    import ml_dtypes

    y = np.ascontiguousarray(np.asarray(y_pred, dtype=np.float32).reshape(N))
    u_all = np.asarray(u_all, dtype=np.float32).reshape(-1)
    u_pos = np.asarray(u_pos, dtype=np.float32).reshape(-1)
    idx = np.asarray(index_s).astype(np.int64).reshape(-1)[:P]
    ua_ps = u_all[idx]
    up_ps = u_pos[idx]
    f = y[:P]

    y_cols = np.ascontiguousarray(y.reshape(128, 128).T)  # [p, k] = y[k*128 + p]
    y_bf = y[:SA].astype(ml_dtypes.bfloat16)
    y_bc = np.ascontiguousarray(np.broadcast_to(y_bf[None, :], (128, SA)))
    eye3 = np.eye(3, dtype=np.float32)
    ones_col = np.ones((128, 1), dtype=np.float32)

    in_maps = []
    for c in range(NCORES):
        rows = slice(c * RPC, (c + 1) * RPC)
        f_bf = f[rows].astype(ml_dtypes.bfloat16)
        in_maps.append(
            {
                "y_bc": y_bc,
                "f_bc": np.ascontiguousarray(np.broadcast_to(f_bf[None, :], (128, RPC))),
                "y_cols": y_cols,
                "f_cols": np.ascontiguousarray(f[rows].reshape(2, 128).T),
                "ua_cols": np.ascontiguousarray(ua_ps[rows].reshape(2, 128).T),
                "up_cols": np.ascontiguousarray(up_ps[rows].reshape(2, 128).T),
                "eye3": eye3,
                "ones_col": ones_col,
            }
        )
    return in_maps


def kernel(**inputs):
    n_pos = int(np.asarray(inputs["n_pos"]))
    assert n_pos == P, f"kernel hardcodes n_pos={P}, got {n_pos}"
    in_maps = make_in_maps(
        inputs["y_pred"], inputs["u_all"], inputs["u_pos"], inputs["index_s"], n_pos
    )
    from concourse.bass_utils import run_bass_kernel_spmd

    nc = get_nc()
    res = run_bass_kernel_spmd(nc, in_maps, list(range(NCORES)))
    total = 0.0
    for r in res.results:
        total += float(r["out"][0, 0])
    return np.float32(total)


# revision 19
# speedup vs baseline: 1.1332x; 1.1332x over previous
"""Trainium2 Bass kernel for nn_APLoss (8 NeuronCores, SPMD row-sharded).

The reference loss collapses to per-row quantities: with c_i = 1 - y_pred[i],
s_ij = relu(c_i + y_pred[j])^2,  R_i = sum_j s_ij,  Rpos_i = sum_{j<2048} s_ij:

  u_a = 0.01*u_all[i] + 0.99*R_i/n        u_p = 0.01*u_pos[i] + 0.99*Rpos_i/n
  loss = mean_i[ (u_p * R_i/n)/u_a^2 - (Rpos_i/n)/u_a ]

Each core owns 256 of the 2048 positive rows (2 partition blocks), holds all of
y_pred, computes its partial sum of the per-row terms; the host sums the 8
partial scalars (the unshard step for a row-sharded scalar mean).

Per-core device pipeline:
  * ScalarE, j in [0, SA): T = Relu(y_j + c_i) with per-partition bias, then
    Square with accum_out -> row sums, split at j=2048 so Rpos falls out.
  * VectorE, j-chunks in [SA, 16384): B[j, r] = (f_r - (y_j+1) < 0) via one
    bf16 tensor_scalar per 128-j chunk (2 elem/cycle/lane).
  * TensorE: (K,S1,S2) = W_chunk^T @ B accumulated in PSUM over chunks (bf16
    in, fp32 acc), W = [1, y, y^2]; then R_tail = (c*K + 2*S1)*c + S2.
  * Small fp32 vector ops do the u-update / p / mean math; PE ones-matmul
    reduces over partitions; each core DMAs out one scalar.

Host-side prep is layout only: dtype casts, reshapes/transposes, the gather
u[index_s[:n_pos]], and replicating y/f slices across the 128 partitions.
"""

import numpy as np

try:
    import concourse.bass as bass  # noqa: F401
except ImportError:  # pragma: no cover
    import sys

    sys.path.insert(0, "/opt/trn_rl_repo")

N = 16384
P = 2048
NCORES = 8
RPC = P // NCORES  # 256 rows per core = 2 partition blocks
SA = 4096  # ScalarE covers j in [0, SA)
K0 = SA // 128  # first indicator j-chunk
ND = (N - SA) // 128  # number of indicator j-chunks
GAMMA = 0.99

_NC_CACHE = {}


def _build_nc():
    import concourse.tile as tile
    from concourse import bacc, mybir

    f32 = mybir.dt.float32
    bf16 = mybir.dt.bfloat16
    Alu = mybir.AluOpType
    Act = mybir.ActivationFunctionType

    nc = bacc.Bacc("TRN2", target_bir_lowering=False, debug=False, num_devices=NCORES)

    y_bc = nc.dram_tensor("y_bc", [128, SA], bf16, kind="ExternalInput").ap()
    f_bfrow = nc.dram_tensor("f_bfrow", [1, RPC], bf16, kind="ExternalInput").ap()
    ones_bfrow = nc.dram_tensor("ones_bfrow", [1, 128], bf16, kind="ExternalInput").ap()
    y_cols = nc.dram_tensor("y_cols", [128, 128], f32, kind="ExternalInput").ap()
    f_cols = nc.dram_tensor("f_cols", [128, 2], f32, kind="ExternalInput").ap()
    ua_cols = nc.dram_tensor("ua_cols", [128, 2], f32, kind="ExternalInput").ap()
    up_cols = nc.dram_tensor("up_cols", [128, 2], f32, kind="ExternalInput").ap()
    eye3 = nc.dram_tensor("eye3", [3, 3], f32, kind="ExternalInput").ap()
    ones_col = nc.dram_tensor("ones_col", [128, 1], f32, kind="ExternalInput").ap()
    out = nc.dram_tensor("out", [1, 1], f32, kind="ExternalOutput").ap()

    with tile.TileContext(nc) as tc:
        with (
            tc.tile_pool(name="const", bufs=1) as cpool,
            tc.tile_pool(name="bpool", bufs=8) as bpool,
            tc.tile_pool(name="scratch", bufs=2) as spool,
            tc.tile_pool(name="small", bufs=1) as mpool,
            tc.tile_pool(name="psum", bufs=1, space="PSUM") as ppool,
        ):
            # Tiny first loads: f row + ones row (~1KB) on sync, then the
            # f broadcast is built by a PE ones-matmul (faster than DMAing a
            # 64KB broadcast through the shared SDMA engines).
            frow = cpool.tile([1, RPC], bf16)
            nc.sync.dma_start(frow[:], f_bfrow[:])
            onesrow = cpool.tile([1, 128], bf16)
            nc.sync.dma_start(onesrow[:], ones_bfrow[:])
            ycols = cpool.tile([128, 128], f32)
            nc.sync.dma_start(ycols[:], y_cols[:])
            psumFb = ppool.tile([128, RPC], f32)
            nc.tensor.matmul(psumFb[:], onesrow[:], frow[:], start=True, stop=True)
            Fb = cpool.tile([128, RPC], bf16)
            nc.vector.tensor_copy(Fb[:], psumFb[:])
            # WAW gate: write a corner of Yb from ycols so the big y broadcast
            # DMAs cannot be scheduled before the gating transfer finishes
            # (the SDMA engines round-robin all queued work, which would
            # otherwise starve ycols).
            Yb = cpool.tile([128, SA], bf16)
            nc.scalar.copy(Yb[0:1, 1024:1025], ycols[0:1, 0:1])
            nc.gpsimd.dma_start(Yb[:, 0:1024], y_bc[:, 0:1024])
            nc.scalar.dma_start(Yb[:, 1024:2048], y_bc[:, 1024:2048])
            nc.gpsimd.dma_start(Yb[:, 2048 : (2048 + SA) // 2], y_bc[:, 2048 : (2048 + SA) // 2])
            nc.scalar.dma_start(Yb[:, (2048 + SA) // 2 : SA], y_bc[:, (2048 + SA) // 2 : SA])
            fcols = cpool.tile([128, 2], f32)
            nc.gpsimd.dma_start(fcols[:], f_cols[:])
            ua = cpool.tile([128, 2], f32)
            nc.scalar.dma_start(ua[:], ua_cols[:])
            up = cpool.tile([128, 2], f32)
            nc.scalar.dma_start(up[:], up_cols[:])
            eye3t = cpool.tile([3, 3], f32)
            nc.gpsimd.dma_start(eye3t[:], eye3[:])
            onest = cpool.tile([128, 1], f32)
            nc.gpsimd.dma_start(onest[:], ones_col[:])

            # y + 1 per-chunk bias columns; c = 1 - f per-block bias columns
            y1 = cpool.tile([128, 128], f32)
            nc.vector.tensor_scalar_add(y1[:], ycols[:], 1.0)
            c_cols = cpool.tile([128, 2], f32)
            nc.scalar.activation(c_cols[:], fcols[:], Act.Identity, bias=1.0, scale=-1.0)

            # W[:, t, :] = [1, y, y^2] in bf16 for chunk k = K0 + t
            W = cpool.tile([128, ND, 3], bf16)
            nc.scalar.activation(W[:, :, 0], ycols[:, K0:128], Act.Copy, bias=1.0, scale=0.0)
            nc.vector.tensor_copy(W[:, :, 1], ycols[:, K0:128])
            nc.scalar.activation(W[:, :, 2], ycols[:, K0:128], Act.Square)

            # --- VectorE + TensorE indicator path: j-chunks [K0, 128) ---
            psumQ = ppool.tile([3, RPC], f32)
            for t in range(ND):
                k = K0 + t
                Bt = bpool.tile([128, RPC], bf16, tag="bt")
                nc.vector.tensor_scalar(
                    Bt[:], Fb[:], y1[:, k : k + 1], 0.0, Alu.subtract, Alu.is_lt
                )
                nc.tensor.matmul(
                    psumQ[:], W[:, t, :], Bt[:], start=(t == 0), stop=(t == ND - 1)
                )

            # --- ScalarE path: j in [0, SA), split at the positive boundary ---
            accPos = mpool.tile([128, 2], f32)
            accRest = mpool.tile([128, 2], f32)
            for b in range(2):
                cb = c_cols[:, b : b + 1]
                t1 = spool.tile([128, P], f32, tag="t1")
                nc.scalar.activation(t1[:], Yb[:, 0:P], Act.Relu, bias=cb)
                t2 = spool.tile([128, P], f32, tag="t2")
                nc.scalar.activation(
                    t2[:], t1[:], Act.Square, accum_out=accPos[:, b : b + 1]
                )
                t3 = spool.tile([128, SA - P], f32, tag="t3")
                nc.scalar.activation(t3[:], Yb[:, P:SA], Act.Relu, bias=cb)
                t4 = spool.tile([128, SA - P], f32, tag="t4")
                nc.scalar.activation(
                    t4[:], t3[:], Act.Square, accum_out=accRest[:, b : b + 1]
                )

            # transpose (3, 256) -> per-row (128, 2, 3) via PE with identity;
            # group a is ready halfway through the chunk loop.
            sbQ = mpool.tile([3, RPC], f32)
            nc.scalar.copy(sbQ[:], psumQ[:])
            psumT = ppool.tile([128, 2, 3], f32)
            for h in range(2):
                nc.tensor.matmul(
                    psumT[:, h, :], sbQ[:, h * 128 : (h + 1) * 128], eye3t[:],
                    start=True, stop=True,
                )
            Kq = psumT[:, :, 0]
            S1q = psumT[:, :, 1]
            S2q = psumT[:, :, 2]

            # R_tail = (c*K + 2*S1)*c + S2
            w1 = mpool.tile([128, 2], f32)
            nc.vector.tensor_tensor(w1[:], c_cols[:], Kq, Alu.mult)
            w2 = mpool.tile([128, 2], f32)
            nc.vector.scalar_tensor_tensor(w2[:], S1q, 2.0, w1[:], Alu.mult, Alu.add)
            w3 = mpool.tile([128, 2], f32)
            nc.vector.tensor_tensor(w3[:], w2[:], c_cols[:], Alu.mult)
            Rt = mpool.tile([128, 2], f32)
            nc.vector.tensor_tensor(Rt[:], w3[:], S2q, Alu.add)

            Ra = mpool.tile([128, 2], f32)
            nc.vector.tensor_tensor(Ra[:], accPos[:], accRest[:], Alu.add)
            R = mpool.tile([128, 2], f32)
            nc.vector.tensor_tensor(R[:], Ra[:], Rt[:], Alu.add)

            # u updates, p, and the mean
            uas = mpool.tile([128, 2], f32)
            nc.vector.tensor_scalar_mul(uas[:], ua[:], 1.0 - GAMMA)
            ups = mpool.tile([128, 2], f32)
            nc.vector.tensor_scalar_mul(ups[:], up[:], 1.0 - GAMMA)
            uan = mpool.tile([128, 2], f32)
            nc.vector.scalar_tensor_tensor(uan[:], R[:], GAMMA / N, uas[:], Alu.mult, Alu.add)
            upn = mpool.tile([128, 2], f32)
            nc.vector.scalar_tensor_tensor(
                upn[:], accPos[:], GAMMA / N, ups[:], Alu.mult, Alu.add
            )
            inv = mpool.tile([128, 2], f32)
            nc.vector.reciprocal(inv[:], uan[:])
            g1 = mpool.tile([128, 2], f32)
            nc.vector.tensor_tensor(g1[:], upn[:], R[:], Alu.mult)
            g2 = mpool.tile([128, 2], f32)
            nc.vector.tensor_tensor(g2[:], g1[:], inv[:], Alu.mult)
            g3 = mpool.tile([128, 2], f32)
            nc.vector.tensor_tensor(g3[:], g2[:], accPos[:], Alu.subtract)
            g4 = mpool.tile([128, 2], f32)
            nc.vector.tensor_tensor(g4[:], g3[:], inv[:], Alu.mult)
            acc = mpool.tile([128, 1], f32)
            nc.vector.tensor_reduce(acc[:], g4[:], mybir.AxisListType.X, Alu.add)
            psumF = ppool.tile([1, 1], f32)
            nc.tensor.matmul(psumF[:], onest[:], acc[:], start=True, stop=True)
            outsb = mpool.tile([1, 1], f32)
            # fold the 1/(N*P) mean normalization into the final copy
            nc.scalar.mul(outsb[:], psumF[:], 1.0 / (float(N) * float(P)))
            nc.sync.dma_start(out[:], outsb[:])

    nc.compile()
    return nc


def get_nc():
    if "nc" not in _NC_CACHE:
        _NC_CACHE["nc"] = _build_nc()
    return _NC_CACHE["nc"]


def make_in_maps(y_pred, u_all, u_pos, index_s, n_pos):
    import ml_dtypes

    y = np.ascontiguousarray(np.asarray(y_pred, dtype=np.float32).reshape(N))
    u_all = np.asarray(u_all, dtype=np.float32).reshape(-1)
    u_pos = np.asarray(u_pos, dtype=np.float32).reshape(-1)
    idx = np.asarray(index_s).astype(np.int64).reshape(-1)[:P]
    ua_ps = u_all[idx]
    up_ps = u_pos[idx]
    f = y[:P]

    y_cols = np.ascontiguousarray(y.reshape(128, 128).T)  # [p, k] = y[k*128 + p]
    y_bf = y[:SA].astype(ml_dtypes.bfloat16)
    y_bc = np.ascontiguousarray(np.broadcast_to(y_bf[None, :], (128, SA)))
    eye3 = np.eye(3, dtype=np.float32)
    ones_col = np.ones((128, 1), dtype=np.float32)

    in_maps = []
    for c in range(NCORES):
        rows = slice(c * RPC, (c + 1) * RPC)
        f_bf = f[rows].astype(ml_dtypes.bfloat16)
        in_maps.append(
            {
                "y_bc": y_bc,
                "f_bfrow": np.ascontiguousarray(f_bf.reshape(1, RPC)),
                "ones_bfrow": np.ones((1, 128), dtype=ml_dtypes.bfloat16),
                "y_cols": y_cols,
                "f_cols": np.ascontiguousarray(f[rows].reshape(2, 128).T),
                "ua_cols": np.ascontiguousarray(ua_ps[rows].reshape(2, 128).T),
                "up_cols": np.ascontiguousarray(up_ps[rows].reshape(2, 128).T),
                "eye3": eye3,
                "ones_col": ones_col,
            }
        )
    return in_maps


def kernel(**inputs):
    n_pos = int(np.asarray(inputs["n_pos"]))
    assert n_pos == P, f"kernel hardcodes n_pos={P}, got {n_pos}"
    in_maps = make_in_maps(
        inputs["y_pred"], inputs["u_all"], inputs["u_pos"], inputs["index_s"], n_pos
    )
    from concourse.bass_utils import run_bass_kernel_spmd

    nc = get_nc()
    res = run_bass_kernel_spmd(nc, in_maps, list(range(NCORES)))
    total = 0.0
    for r in res.results:
        total += float(r["out"][0, 0])
    return np.float32(total)


# revision 20
# speedup vs baseline: 1.2184x; 1.0752x over previous
"""Trainium2 Bass kernel for nn_APLoss (8 NeuronCores, SPMD row-sharded).

The reference loss collapses to per-row quantities: with c_i = 1 - y_pred[i],
s_ij = relu(c_i + y_pred[j])^2,  R_i = sum_j s_ij,  Rpos_i = sum_{j<2048} s_ij:

  u_a = 0.01*u_all[i] + 0.99*R_i/n        u_p = 0.01*u_pos[i] + 0.99*Rpos_i/n
  loss = mean_i[ (u_p * R_i/n)/u_a^2 - (Rpos_i/n)/u_a ]

Each core owns 256 of the 2048 positive rows (2 partition blocks), holds all of
y_pred, computes its partial sum of the per-row terms; the host sums the 8
partial scalars (the unshard step for a row-sharded scalar mean).

Per-core device pipeline:
  * ScalarE, j in [0, SA): T = Relu(y_j + c_i) with per-partition bias, then
    Square with accum_out -> row sums, split at j=2048 so Rpos falls out.
  * VectorE, j-chunks in [SA, 16384): B[j, r] = (f_r - (y_j+1) < 0) via one
    bf16 tensor_scalar per 128-j chunk (2 elem/cycle/lane).
  * TensorE: (K,S1,S2) = W_chunk^T @ B accumulated in PSUM over chunks (bf16
    in, fp32 acc), W = [1, y, y^2]; then R_tail = (c*K + 2*S1)*c + S2.
  * Small fp32 vector ops do the u-update / p / mean math; PE ones-matmul
    reduces over partitions; each core DMAs out one scalar.

Host-side prep is layout only: dtype casts, reshapes/transposes, the gather
u[index_s[:n_pos]], and replicating y/f slices across the 128 partitions.
"""

import numpy as np

try:
    import concourse.bass as bass  # noqa: F401
except ImportError:  # pragma: no cover
    import sys

    sys.path.insert(0, "/opt/trn_rl_repo")

N = 16384
P = 2048
NCORES = 8
RPC = P // NCORES  # 256 rows per core = 2 partition blocks
SA = 4096  # ScalarE covers j in [0, SA)
K0 = SA // 128  # first indicator j-chunk
ND = (N - SA) // 128  # number of indicator j-chunks
GAMMA = 0.99

_NC_CACHE = {}


def _build_nc():
    import concourse.tile as tile
    from concourse import bacc, mybir

    f32 = mybir.dt.float32
    bf16 = mybir.dt.bfloat16
    Alu = mybir.AluOpType
    Act = mybir.ActivationFunctionType

    nc = bacc.Bacc("TRN2", target_bir_lowering=False, debug=False, num_devices=NCORES)

    y_bc = nc.dram_tensor("y_bc", [128, SA], bf16, kind="ExternalInput").ap()
    f_bfrow = nc.dram_tensor("f_bfrow", [1, RPC], bf16, kind="ExternalInput").ap()
    ones_bfrow = nc.dram_tensor("ones_bfrow", [1, 128], bf16, kind="ExternalInput").ap()
    y_cols = nc.dram_tensor("y_cols", [128, 128], f32, kind="ExternalInput").ap()
    f_cols = nc.dram_tensor("f_cols", [128, 2], f32, kind="ExternalInput").ap()
    ua_cols = nc.dram_tensor("ua_cols", [128, 2], f32, kind="ExternalInput").ap()
    up_cols = nc.dram_tensor("up_cols", [128, 2], f32, kind="ExternalInput").ap()
    eye3 = nc.dram_tensor("eye3", [3, 3], f32, kind="ExternalInput").ap()
    ones_col = nc.dram_tensor("ones_col", [128, 1], f32, kind="ExternalInput").ap()
    out = nc.dram_tensor("out", [1, 1], f32, kind="ExternalOutput").ap()

    with tile.TileContext(nc) as tc:
        with (
            tc.tile_pool(name="const", bufs=1) as cpool,
            tc.tile_pool(name="bpool", bufs=8) as bpool,
            tc.tile_pool(name="scratch", bufs=2) as spool,
            tc.tile_pool(name="small", bufs=1) as mpool,
            tc.tile_pool(name="psum", bufs=1, space="PSUM") as ppool,
        ):
            # Tiny first loads: f row + ones row (~1KB) on sync, then the
            # f broadcast is built by a PE ones-matmul (faster than DMAing a
            # 64KB broadcast through the shared SDMA engines).
            frow = cpool.tile([1, RPC], bf16)
            nc.sync.dma_start(frow[:], f_bfrow[:])
            onesrow = cpool.tile([1, 128], bf16)
            nc.sync.dma_start(onesrow[:], ones_bfrow[:])
            ycols = cpool.tile([128, 128], f32)
            nc.sync.dma_start(ycols[:], y_cols[:])
            psumFb = ppool.tile([128, RPC], f32)
            nc.tensor.matmul(psumFb[:], onesrow[:], frow[:], start=True, stop=True)
            Fb = cpool.tile([128, RPC], bf16)
            nc.vector.tensor_copy(Fb[:], psumFb[:])
            # WAW gates: write a corner of each Yb quarter from the small
            # gating transfers (frow/ycols) so none of the big y broadcast
            # DMAs can be scheduled before those finish (the SDMA engines
            # round-robin all queued work, which would otherwise starve the
            # small loads that gate all compute).
            Yb = cpool.tile([128, SA], bf16)
            QTR = SA // 4
            for q in range(4):
                if q % 2 == 0:
                    nc.gpsimd.tensor_copy(Yb[0:1, q * QTR : q * QTR + 1], frow[0:1, 0:1])
                else:
                    nc.scalar.copy(Yb[0:1, q * QTR : q * QTR + 1], ycols[0:1, 0:1])
            nc.gpsimd.dma_start(Yb[:, 0:QTR], y_bc[:, 0:QTR])
            nc.scalar.dma_start(Yb[:, QTR : 2 * QTR], y_bc[:, QTR : 2 * QTR])
            nc.gpsimd.dma_start(Yb[:, 2 * QTR : 3 * QTR], y_bc[:, 2 * QTR : 3 * QTR])
            nc.scalar.dma_start(Yb[:, 3 * QTR : SA], y_bc[:, 3 * QTR : SA])
            fcols = cpool.tile([128, 2], f32)
            nc.gpsimd.dma_start(fcols[:], f_cols[:])
            ua = cpool.tile([128, 2], f32)
            nc.scalar.dma_start(ua[:], ua_cols[:])
            up = cpool.tile([128, 2], f32)
            nc.scalar.dma_start(up[:], up_cols[:])
            eye3t = cpool.tile([3, 3], f32)
            nc.gpsimd.dma_start(eye3t[:], eye3[:])
            onest = cpool.tile([128, 1], f32)
            nc.gpsimd.dma_start(onest[:], ones_col[:])

            # y + 1 per-chunk bias columns; c = 1 - f per-block bias columns
            y1 = cpool.tile([128, 128], f32)
            nc.vector.tensor_scalar_add(y1[:], ycols[:], 1.0)
            c_cols = cpool.tile([128, 2], f32)
            nc.scalar.activation(c_cols[:], fcols[:], Act.Identity, bias=1.0, scale=-1.0)

            # W[:, t, :] = [1, y, y^2] in bf16 for chunk k = K0 + t
            W = cpool.tile([128, ND, 3], bf16)
            nc.scalar.activation(W[:, :, 0], ycols[:, K0:128], Act.Copy, bias=1.0, scale=0.0)
            nc.vector.tensor_copy(W[:, :, 1], ycols[:, K0:128])
            nc.scalar.activation(W[:, :, 2], ycols[:, K0:128], Act.Square)

            # --- VectorE + TensorE indicator path: j-chunks [K0, 128) ---
            psumQ = ppool.tile([3, RPC], f32)
            for t in range(ND):
                k = K0 + t
                Bt = bpool.tile([128, RPC], bf16, tag="bt")
                nc.vector.tensor_scalar(
                    Bt[:], Fb[:], y1[:, k : k + 1], 0.0, Alu.subtract, Alu.is_lt
                )
                nc.tensor.matmul(
                    psumQ[:], W[:, t, :], Bt[:], start=(t == 0), stop=(t == ND - 1)
                )

            # --- ScalarE path: j in [0, SA), split at the positive boundary ---
            accPos = mpool.tile([128, 2], f32)
            accRest = mpool.tile([128, 2], f32)
            for b in range(2):
                cb = c_cols[:, b : b + 1]
                t1 = spool.tile([128, P], f32, tag="t1")
                nc.scalar.activation(t1[:], Yb[:, 0:P], Act.Relu, bias=cb)
                t2 = spool.tile([128, P], f32, tag="t2")
                nc.scalar.activation(
                    t2[:], t1[:], Act.Square, accum_out=accPos[:, b : b + 1]
                )
                t3 = spool.tile([128, SA - P], f32, tag="t3")
                nc.scalar.activation(t3[:], Yb[:, P:SA], Act.Relu, bias=cb)
                t4 = spool.tile([128, SA - P], f32, tag="t4")
                nc.scalar.activation(
                    t4[:], t3[:], Act.Square, accum_out=accRest[:, b : b + 1]
                )

            # transpose (3, 256) -> per-row (128, 2, 3) via PE with identity;
            # group a is ready halfway through the chunk loop.
            sbQ = mpool.tile([3, RPC], f32)
            nc.scalar.copy(sbQ[:], psumQ[:])
            psumT = ppool.tile([128, 2, 3], f32)
            for h in range(2):
                nc.tensor.matmul(
                    psumT[:, h, :], sbQ[:, h * 128 : (h + 1) * 128], eye3t[:],
                    start=True, stop=True,
                )
            Kq = psumT[:, :, 0]
            S1q = psumT[:, :, 1]
            S2q = psumT[:, :, 2]

            # R_tail = (c*K + 2*S1)*c + S2
            w1 = mpool.tile([128, 2], f32)
            nc.vector.tensor_tensor(w1[:], c_cols[:], Kq, Alu.mult)
            w2 = mpool.tile([128, 2], f32)
            nc.vector.scalar_tensor_tensor(w2[:], S1q, 2.0, w1[:], Alu.mult, Alu.add)
            w3 = mpool.tile([128, 2], f32)
            nc.vector.tensor_tensor(w3[:], w2[:], c_cols[:], Alu.mult)
            Rt = mpool.tile([128, 2], f32)
            nc.vector.tensor_tensor(Rt[:], w3[:], S2q, Alu.add)

            Ra = mpool.tile([128, 2], f32)
            nc.vector.tensor_tensor(Ra[:], accPos[:], accRest[:], Alu.add)
            R = mpool.tile([128, 2], f32)
            nc.vector.tensor_tensor(R[:], Ra[:], Rt[:], Alu.add)

            # u updates, p, and the mean
            uas = mpool.tile([128, 2], f32)
            nc.vector.tensor_scalar_mul(uas[:], ua[:], 1.0 - GAMMA)
            ups = mpool.tile([128, 2], f32)
            nc.vector.tensor_scalar_mul(ups[:], up[:], 1.0 - GAMMA)
            uan = mpool.tile([128, 2], f32)
            nc.vector.scalar_tensor_tensor(uan[:], R[:], GAMMA / N, uas[:], Alu.mult, Alu.add)
            upn = mpool.tile([128, 2], f32)
            nc.vector.scalar_tensor_tensor(
                upn[:], accPos[:], GAMMA / N, ups[:], Alu.mult, Alu.add
            )
            inv = mpool.tile([128, 2], f32)
            nc.vector.reciprocal(inv[:], uan[:])
            g1 = mpool.tile([128, 2], f32)
            nc.vector.tensor_tensor(g1[:], upn[:], R[:], Alu.mult)
            g2 = mpool.tile([128, 2], f32)
            nc.vector.tensor_tensor(g2[:], g1[:], inv[:], Alu.mult)
            g3 = mpool.tile([128, 2], f32)
            nc.vector.tensor_tensor(g3[:], g2[:], accPos[:], Alu.subtract)
            g4 = mpool.tile([128, 2], f32)
            nc.vector.tensor_tensor(g4[:], g3[:], inv[:], Alu.mult)
            acc = mpool.tile([128, 1], f32)
            nc.vector.tensor_reduce(acc[:], g4[:], mybir.AxisListType.X, Alu.add)
            psumF = ppool.tile([1, 1], f32)
            nc.tensor.matmul(psumF[:], onest[:], acc[:], start=True, stop=True)
            outsb = mpool.tile([1, 1], f32)
            # fold the 1/(N*P) mean normalization into the final copy
            nc.scalar.mul(outsb[:], psumF[:], 1.0 / (float(N) * float(P)))
            nc.sync.dma_start(out[:], outsb[:])

    nc.compile()
    return nc


def get_nc():
    if "nc" not in _NC_CACHE:
        _NC_CACHE["nc"] = _build_nc()
    return _NC_CACHE["nc"]


def make_in_maps(y_pred, u_all, u_pos, index_s, n_pos):
    import ml_dtypes

    y = np.ascontiguousarray(np.asarray(y_pred, dtype=np.float32).reshape(N))
    u_all = np.asarray(u_all, dtype=np.float32).reshape(-1)
    u_pos = np.asarray(u_pos, dtype=np.float32).reshape(-1)
    idx = np.asarray(index_s).astype(np.int64).reshape(-1)[:P]
    ua_ps = u_all[idx]
    up_ps = u_pos[idx]
    f = y[:P]

    y_cols = np.ascontiguousarray(y.reshape(128, 128).T)  # [p, k] = y[k*128 + p]
    y_bf = y[:SA].astype(ml_dtypes.bfloat16)
    y_bc = np.ascontiguousarray(np.broadcast_to(y_bf[None, :], (128, SA)))
    eye3 = np.eye(3, dtype=np.float32)
    ones_col = np.ones((128, 1), dtype=np.float32)

    in_maps = []
    for c in range(NCORES):
        rows = slice(c * RPC, (c + 1) * RPC)
        f_bf = f[rows].astype(ml_dtypes.bfloat16)
        in_maps.append(
            {
                "y_bc": y_bc,
                "f_bfrow": np.ascontiguousarray(f_bf.reshape(1, RPC)),
                "ones_bfrow": np.ones((1, 128), dtype=ml_dtypes.bfloat16),
                "y_cols": y_cols,
                "f_cols": np.ascontiguousarray(f[rows].reshape(2, 128).T),
                "ua_cols": np.ascontiguousarray(ua_ps[rows].reshape(2, 128).T),
                "up_cols": np.ascontiguousarray(up_ps[rows].reshape(2, 128).T),
                "eye3": eye3,
                "ones_col": ones_col,
            }
        )
    return in_maps


def kernel(**inputs):
    n_pos = int(np.asarray(inputs["n_pos"]))
    assert n_pos == P, f"kernel hardcodes n_pos={P}, got {n_pos}"
    in_maps = make_in_maps(
        inputs["y_pred"], inputs["u_all"], inputs["u_pos"], inputs["index_s"], n_pos
    )
    from concourse.bass_utils import run_bass_kernel_spmd

    nc = get_nc()
    res = run_bass_kernel_spmd(nc, in_maps, list(range(NCORES)))
    total = 0.0
    for r in res.results:
        total += float(r["out"][0, 0])
    return np.float32(total)
